# revision 11
# baseline (speedup 1.0000x reference)
"""Trainium2 Bass kernel for nn_CompactControlAttention.

The module's attention is degenerate: softmax over a size-1 axis is exactly
1.0, so queries/keys (Wq, bq, Wk, bk) never affect the output:

    out[b, s, :] = sequence[b, s, :] + p[b, :]
    p = (sum_c controls[c]) @ Wv.T @ Wo.T + C * (bv @ Wo.T + bo)

Sharding (8 cores, no collectives -- collectives cost ~75us of fixed setup
per execute on this stack): output-feature parallel. Core k owns e-slice
ek (256 cols): it computes the full v = cs @ Wv.T (streamed against the
Wv.T DMA), then p[:, ek] = v @ Wo.T[:, ek].

Per-core device program (everything bf16 except PSUM/output):
  cs_t = sum_c controls_t[c]     (controls shipped pre-transposed [d, c*b];
                                  DVE tree sum -> [d partitions, b free])
  v    = cs @ Wv.T + C*bv        (64 matmuls, 512-wide moving, k-streamed)
  vt   = v.T                     (16 PE transposes)
  p    = v @ Wo.T[:, ek] + bo    (16 matmuls, 256-wide moving)
  out  = seq_k + broadcast_s(p)  (free-dim step-0 broadcast on DVE/GpSimd)
"""

import numpy as np
import ml_dtypes

import concourse.bass as bass
import concourse.mybir as mybir
import concourse.tile as tile
from concourse import bacc
from concourse.bass_utils import run_bass_kernel_spmd
from concourse.masks import make_identity

N_CORES = 8
D = 2048
B = 64
S = 32
C = 8
EK = D // N_CORES  # 256: output-feature slice per core
NT = D // 128  # 16 contraction tiles
F32 = mybir.dt.float32
BF16 = mybir.dt.bfloat16

_CACHE = {}


def _build_nc():
    nc = bacc.Bacc("TRN2", target_bir_lowering=False, debug=False, num_devices=N_CORES)

    ctrl = nc.dram_tensor("ctrl", [D, C * B], BF16, kind="ExternalInput")  # controls^T
    wvt = nc.dram_tensor("wvt", [D, D], BF16, kind="ExternalInput")  # Wv.T
    wot = nc.dram_tensor("wot", [D, EK], BF16, kind="ExternalInput")  # Wo.T[:, ek]
    bv = nc.dram_tensor("bv", [D], F32, kind="ExternalInput")
    bo = nc.dram_tensor("bo", [EK], F32, kind="ExternalInput")
    seq = nc.dram_tensor("seq", [128, S * 128], BF16, kind="ExternalInput")
    out = nc.dram_tensor("out", [128, S * 128], F32, kind="ExternalOutput")

    with tile.TileContext(nc) as tc:
        _body(tc, ctrl, wvt, wot, bv, bo, seq, out)
    nc.compile()
    return nc


def _body(tc, ctrl, wvt, wot, bv, bo, seq, out):
    from contextlib import ExitStack

    ctx = ExitStack()
    nc = tc.nc
    P = 128

    consts = ctx.enter_context(tc.tile_pool(name="consts", bufs=1))
    sbuf = ctx.enter_context(tc.tile_pool(name="sbuf", bufs=1))
    wpool = ctx.enter_context(tc.tile_pool(name="wv", bufs=3))
    psum_v = ctx.enter_context(tc.tile_pool(name="psum_v", bufs=1, space="PSUM"))
    psum_t = ctx.enter_context(tc.tile_pool(name="psum_t", bufs=2, space="PSUM"))
    psum_p = ctx.enter_context(tc.tile_pool(name="psum_p", bufs=1, space="PSUM"))

    # --- ctrl first on both queues: the cs chain heads the critical path ---
    ctrl_sb = sbuf.tile([P, NT * C * B], BF16)  # [p, t, c, b]
    c3 = ctrl_sb[:].rearrange("p (t cb) -> p t cb", cb=C * B)
    nc.sync.dma_start(
        out=c3[:, 0 : NT // 2, :],
        in_=ctrl[0 : D // 2, :].rearrange("(t p) cb -> p t cb", p=P),
    )
    nc.scalar.dma_start(
        out=c3[:, NT // 2 : NT, :],
        in_=ctrl[D // 2 : D, :].rearrange("(t p) cb -> p t cb", p=P),
    )

    # --- constants ---
    ident = consts.tile([P, P], F32)
    make_identity(nc, ident[:])
    ident_b = consts.tile([P, P], BF16)
    nc.vector.tensor_copy(ident_b[:], ident[:])
    ones8_f = consts.tile([1, B], F32)
    nc.vector.memset(ones8_f[:], float(C))
    ones8 = consts.tile([1, B], BF16)  # value C: bias-augment row for MM1
    nc.vector.tensor_copy(ones8[:], ones8_f[:])
    ones1_f = consts.tile([1, B], F32)
    nc.vector.memset(ones1_f[:], 1.0)
    ones1 = consts.tile([1, B], BF16)  # value 1: bias-augment row for MM2
    nc.vector.tensor_copy(ones1[:], ones1_f[:])

    # --- cs = sum_c controls: bf16 tree sum per t-half as each half lands --
    c4 = ctrl_sb[:].rearrange("p (t c b) -> p t c b", c=C, b=B)
    s1 = sbuf.tile([P, NT * 4 * B], BF16)
    s1v = s1[:].rearrange("p (t c b) -> p t c b", c=4, b=B)
    s2 = sbuf.tile([P, NT * 2 * B], BF16)
    s2v = s2[:].rearrange("p (t c b) -> p t c b", c=2, b=B)
    cs = sbuf.tile([P, NT * B], BF16)
    csv = cs[:].rearrange("p (t b) -> p t b", b=B)
    H = NT // 2
    for ts in (slice(0, H), slice(H, NT)):
        nc.vector.tensor_add(s1v[:, ts], c4[:, ts, 0:4, :], c4[:, ts, 4:8, :])
        nc.vector.tensor_add(s2v[:, ts], s1v[:, ts, 0:2, :], s1v[:, ts, 2:4, :])
        nc.vector.tensor_add(csv[:, ts], s2v[:, ts, 0, :], s2v[:, ts, 1, :])

    # --- MM1: v = cs @ Wv.T + C*bv  (v in 4 PSUM banks of [64, 512]) ------
    # Wv.T streamed in 8 chunks of 2 k-tiles (0.5MB each), alternating
    # queues; bufs=3 keeps the DMA 2-3 chunks ahead of the PE.
    pv = [psum_v.tile([B, 512], F32, tag=f"pv{c}", name=f"pv{c}") for c in range(4)]
    for jj in range(8):
        wv_sb = wpool.tile([P, 2 * D], BF16)
        q = nc.sync if jj % 2 == 0 else nc.scalar
        q.dma_start(
            out=wv_sb[:].rearrange("p (g d) -> p g d", d=D),
            in_=wvt[jj * 256 : (jj + 1) * 256, :].rearrange("(g p) d -> p g d", p=P),
        )
        wv3 = wv_sb[:].rearrange("p (g d) -> p g d", d=D)
        for g in range(2):
            j = 2 * jj + g
            for c in range(4):
                nc.tensor.matmul(
                    pv[c][:],
                    csv[:, j, :],
                    wv3[:, g, c * 512 : (c + 1) * 512],
                    start=(j == 0),
                    stop=False,
                )

    # --- late inputs, after the wv stream in each queue FIFO ---
    bv_sb = consts.tile([1, D], F32)
    nc.gpsimd.dma_start(out=bv_sb[:], in_=bv[None, :])
    bo_sb = consts.tile([1, EK], F32)
    nc.gpsimd.dma_start(out=bo_sb[:], in_=bo[None, :])
    wot_sb = sbuf.tile([P, NT * EK], BF16)  # [p, t, e]
    nc.scalar.dma_start(
        out=wot_sb[:].rearrange("p (t e) -> p t e", e=EK),
        in_=wot.rearrange("(t p) e -> p t e", p=P),
    )
    seq_sb = sbuf.tile([P, S * 128], BF16)
    nc.sync.dma_start(out=seq_sb[:], in_=seq[:])
    bv_b = consts.tile([1, D], BF16)
    nc.vector.tensor_copy(bv_b[:], bv_sb[:])
    bo_b = consts.tile([1, EK], BF16)
    nc.vector.tensor_copy(bo_b[:], bo_sb[:])

    for c in range(4):  # bias-augment row: += C * bv
        nc.tensor.matmul(
            pv[c][:], ones8[:], bv_b[:, c * 512 : (c + 1) * 512],
            start=False, stop=True,
        )
    v = sbuf.tile([B, D], BF16)
    for c in range(4):
        nc.vector.tensor_copy(v[:, c * 512 : (c + 1) * 512], pv[c][:])

    # --- vt = v.T via 16 PE transposes -> [p, t, b] with f = t*128 + p ----
    vt = sbuf.tile([P, NT * B], BF16)
    vt3 = vt[:].rearrange("p (t b) -> p t b", b=B)
    for t in range(NT):
        pt = psum_t.tile([P, B], BF16, tag="pt", name="pt")
        nc.tensor.transpose(pt[:], v[:, t * P : (t + 1) * P], ident_b[0:B, 0:B])
        nc.vector.tensor_copy(vt3[:, t, :], pt[:])

    # --- MM2: p = v @ Wo.T[:, ek] + bo ------------------------------------
    pp = psum_p.tile([B, EK], F32, tag="pp")
    wo3 = wot_sb[:].rearrange("p (t e) -> p t e", e=EK)
    for t in range(NT):
        nc.tensor.matmul(
            pp[:], vt3[:, t, :], wo3[:, t, :], start=(t == 0), stop=False
        )
    nc.tensor.matmul(pp[:], ones1[:], bo_b[:], start=False, stop=True)

    # p_re: partition (eh*64 + b), free e' -- matches seq layout
    p_re = sbuf.tile([P, P], F32)
    nc.vector.tensor_copy(p_re[0:B, :], pp[:, 0:P])
    nc.vector.tensor_copy(p_re[B : 2 * B, :], pp[:, P : 2 * P])

    # --- out = seq + broadcast_s(p): chunks pipelined with the out DMA ----
    # DVE is ~2.5x faster than GpSimd at this add; split 24/8 rows and
    # pipeline each chunk's DMA on the two HWDGE queues.
    out_sb = sbuf.tile([P, S * 128], F32)
    o3 = out_sb[:].rearrange("p (s e) -> p s e", e=P)
    q3 = seq_sb[:].rearrange("p (s e) -> p s e", e=P)
    chunks = [  # (engine, s0, s1, queue)
        (nc.gpsimd, 24, 32, nc.scalar),
        (nc.vector, 0, 12, nc.sync),
        (nc.vector, 12, 24, nc.sync),
    ]
    for eng, s0, s1, q in chunks:
        eng.tensor_add(
            o3[:, s0:s1, :], q3[:, s0:s1, :],
            p_re[:, None, :].to_broadcast((P, s1 - s0, P)),
        )
        q.dma_start(out=out[:, s0 * 128 : s1 * 128], in_=out_sb[:, s0 * 128 : s1 * 128])
    ctx.close()


def _get_nc():
    if "nc" not in _CACHE:
        _CACHE["nc"] = _build_nc()
    return _CACHE["nc"]


def _shard(sequence, controls, Wv, bv, Wo, bo):
    bf = ml_dtypes.bfloat16
    ctrl_t = np.ascontiguousarray(
        controls.transpose(2, 0, 1).reshape(D, C * B).astype(bf)
    )
    wvt = np.ascontiguousarray(Wv.T.astype(bf))
    bvc = np.ascontiguousarray(bv)
    in_maps = []
    for k in range(N_CORES):
        ek = slice(k * EK, (k + 1) * EK)
        in_maps.append(
            {
                "ctrl": ctrl_t,
                "wvt": wvt,
                "wot": np.ascontiguousarray(Wo[ek, :].T.astype(bf)),
                "bv": bvc,
                "bo": np.ascontiguousarray(bo[ek]),
                "seq": np.ascontiguousarray(
                    sequence[:, :, ek]
                    .reshape(B, S, 2, 128)
                    .transpose(2, 0, 1, 3)
                    .reshape(128, S * 128)
                    .astype(bf)
                ),
            }
        )
    return in_maps


def _run(inputs, trace=False):
    nc = _get_nc()
    in_maps = _shard(
        np.asarray(inputs["sequence"]), np.asarray(inputs["controls"]),
        np.asarray(inputs["Wv"]), np.asarray(inputs["bv"]),
        np.asarray(inputs["Wo"]), np.asarray(inputs["bo"]),
    )
    res = run_bass_kernel_spmd(nc, in_maps, list(range(N_CORES)), trace=trace)
    out = np.empty((B, S, D), dtype=np.float32)
    for k in range(N_CORES):
        out[:, :, k * EK : (k + 1) * EK] = (
            res.results[k]["out"]
            .reshape(2, B, S, 128)
            .transpose(1, 2, 0, 3)
            .reshape(B, S, EK)
        )
    return out, res


def kernel(**inputs):
    out, _ = _run(inputs)
    return out


# revision 12
# speedup vs baseline: 1.2795x; 1.2795x over previous
"""Trainium2 Bass kernel for nn_CompactControlAttention.

The module's attention is degenerate: softmax over a size-1 axis is exactly
1.0, so queries/keys (Wq, bq, Wk, bk) never affect the output:

    out[b, s, :] = sequence[b, s, :] + p[b, :]
    p = (sum_c controls[c]) @ Wv.T @ Wo.T + C * (bv @ Wo.T + bo)

Sharding: tensor-parallel over the hidden feature dim f of v = cs @ Wv.T
and over the output feature dim e of p. Cross-core exchange of the tiny
v.T (256KB) happens between two NEFF launches via host gather -- on-chip
collectives cost ~75us of fixed setup per execute on this stack, and HBM
is only pair-shared, so a host hop is the cheapest 8-way exchange.

NEFF-1 (per core k, ~3MB DMA):
  cs_t = sum_c controls_t[c]      (controls shipped pre-transposed, bf16)
  v_k  = cs @ Wv.T[:, fk] + C*bv  (16 bf16 matmuls, 256-wide, PSUM accum)
  vt_k = v_k.T                    (2 PE transposes) -> out [256, 64] bf16

NEFF-2 (per core k, ~4.3MB DMA; host feeds the gathered full v.T):
  p_k  = v @ Wo.T[:, ek] + bo     (16 bf16 matmuls)
  out  = seq_k + broadcast_s(p_k) (chunked DVE/GpSimd adds, piped DMA)
"""

import numpy as np
import ml_dtypes

import concourse.bass as bass
import concourse.mybir as mybir
import concourse.tile as tile
from concourse import bacc
from concourse.bass_utils import run_bass_kernel_spmd
from concourse.masks import make_identity

N_CORES = 8
D = 2048
B = 64
S = 32
C = 8
EK = D // N_CORES  # 256
NT = D // 128  # 16
F32 = mybir.dt.float32
BF16 = mybir.dt.bfloat16

_CACHE = {}


# --------------------------- NEFF-1: v.T slice ---------------------------


def _build_nc1():
    nc = bacc.Bacc("TRN2", target_bir_lowering=False, debug=False, num_devices=N_CORES)
    ctrl = nc.dram_tensor("ctrl", [D, C * B], BF16, kind="ExternalInput")
    wvt = nc.dram_tensor("wvt", [D, EK], BF16, kind="ExternalInput")  # Wv.T[:, fk]
    bv = nc.dram_tensor("bv", [EK], F32, kind="ExternalInput")
    vt_out = nc.dram_tensor("vt", [EK, B], BF16, kind="ExternalOutput")

    with tile.TileContext(nc) as tc:
        from contextlib import ExitStack

        ctx = ExitStack()
        P = 128
        consts = ctx.enter_context(tc.tile_pool(name="consts", bufs=1))
        sbuf = ctx.enter_context(tc.tile_pool(name="sbuf", bufs=1))
        psum_v = ctx.enter_context(tc.tile_pool(name="psum_v", bufs=1, space="PSUM"))
        psum_t = ctx.enter_context(tc.tile_pool(name="psum_t", bufs=1, space="PSUM"))

        ctrl_sb = sbuf.tile([P, NT * C * B], BF16)
        c3 = ctrl_sb[:].rearrange("p (t cb) -> p t cb", cb=C * B)
        nc.sync.dma_start(
            out=c3[:, 0 : NT // 2, :],
            in_=ctrl[0 : D // 2, :].rearrange("(t p) cb -> p t cb", p=P),
        )
        nc.scalar.dma_start(
            out=c3[:, NT // 2 : NT, :],
            in_=ctrl[D // 2 : D, :].rearrange("(t p) cb -> p t cb", p=P),
        )
        wv_sb = sbuf.tile([P, NT * EK], BF16)
        nc.sync.dma_start(
            out=wv_sb[:].rearrange("p (t f) -> p t f", f=EK),
            in_=wvt.rearrange("(t p) f -> p t f", p=P),
        )
        bv_sb = consts.tile([1, EK], F32)
        nc.gpsimd.dma_start(out=bv_sb[:], in_=bv[None, :])

        ident = consts.tile([P, P], F32)
        make_identity(nc, ident[:])
        ident_b = consts.tile([P, P], BF16)
        nc.vector.tensor_copy(ident_b[:], ident[:])
        ones8_f = consts.tile([1, B], F32)
        nc.vector.memset(ones8_f[:], float(C))
        ones8 = consts.tile([1, B], BF16)
        nc.vector.tensor_copy(ones8[:], ones8_f[:])
        bv_b = consts.tile([1, EK], BF16)
        nc.vector.tensor_copy(bv_b[:], bv_sb[:])

        # cs tree sum, t-halves as each ctrl half lands
        c4 = ctrl_sb[:].rearrange("p (t c b) -> p t c b", c=C, b=B)
        s1 = sbuf.tile([P, NT * 4 * B], BF16)
        s1v = s1[:].rearrange("p (t c b) -> p t c b", c=4, b=B)
        s2 = sbuf.tile([P, NT * 2 * B], BF16)
        s2v = s2[:].rearrange("p (t c b) -> p t c b", c=2, b=B)
        cs = sbuf.tile([P, NT * B], BF16)
        csv = cs[:].rearrange("p (t b) -> p t b", b=B)
        H = NT // 2
        for ts in (slice(0, H), slice(H, NT)):
            nc.vector.tensor_add(s1v[:, ts], c4[:, ts, 0:4, :], c4[:, ts, 4:8, :])
            nc.vector.tensor_add(s2v[:, ts], s1v[:, ts, 0:2, :], s1v[:, ts, 2:4, :])
            nc.vector.tensor_add(csv[:, ts], s2v[:, ts, 0, :], s2v[:, ts, 1, :])

        # MM1 + bias
        pv = psum_v.tile([B, EK], F32, tag="pv")
        wv3 = wv_sb[:].rearrange("p (t f) -> p t f", f=EK)
        for t in range(NT):
            nc.tensor.matmul(
                pv[:], csv[:, t, :], wv3[:, t, :], start=(t == 0), stop=False
            )
        nc.tensor.matmul(pv[:], ones8[:], bv_b[:], start=False, stop=True)
        v = sbuf.tile([B, EK], BF16)
        nc.vector.tensor_copy(v[:], pv[:])

        # vt = v.T
        pt = psum_t.tile([P, 2 * B], BF16, tag="pt")
        for g in range(2):
            nc.tensor.transpose(
                pt[:, g * B : (g + 1) * B], v[:, g * 128 : (g + 1) * 128],
                ident_b[0:B, 0:B],
            )
        vt = sbuf.tile([P, 2 * B], BF16)
        nc.vector.tensor_copy(vt[:], pt[:])
        nc.sync.dma_start(
            out=vt_out[:].rearrange("(g p) b -> p g b", p=P),
            in_=vt[:].rearrange("p (g b) -> p g b", b=B),
        )
        ctx.close()
    nc.compile()
    return nc


# ------------------------ NEFF-2: MM2 + residual -------------------------


def _build_nc2():
    nc = bacc.Bacc("TRN2", target_bir_lowering=False, debug=False, num_devices=N_CORES)
    vta = nc.dram_tensor("vta", [D, B], BF16, kind="ExternalInput")  # full v.T
    wot = nc.dram_tensor("wot", [D, EK], BF16, kind="ExternalInput")  # Wo.T[:, ek]
    bo = nc.dram_tensor("bo", [EK], F32, kind="ExternalInput")
    seq = nc.dram_tensor("seq", [128, S * 128], BF16, kind="ExternalInput")
    out = nc.dram_tensor("out", [128, S * 128], F32, kind="ExternalOutput")

    with tile.TileContext(nc) as tc:
        from contextlib import ExitStack

        ctx = ExitStack()
        P = 128
        consts = ctx.enter_context(tc.tile_pool(name="consts", bufs=1))
        sbuf = ctx.enter_context(tc.tile_pool(name="sbuf", bufs=1))
        psum_p = ctx.enter_context(tc.tile_pool(name="psum_p", bufs=1, space="PSUM"))

        vta_sb = sbuf.tile([P, NT * B], BF16)
        vta3 = vta_sb[:].rearrange("p (t b) -> p t b", b=B)
        nc.sync.dma_start(out=vta3, in_=vta.rearrange("(t p) b -> p t b", p=P))
        wot_sb = sbuf.tile([P, NT * EK], BF16)
        nc.scalar.dma_start(
            out=wot_sb[:].rearrange("p (t e) -> p t e", e=EK),
            in_=wot.rearrange("(t p) e -> p t e", p=P),
        )
        bo_sb = consts.tile([1, EK], F32)
        nc.gpsimd.dma_start(out=bo_sb[:], in_=bo[None, :])
        seq_sb = sbuf.tile([P, S * 128], BF16)
        nc.sync.dma_start(out=seq_sb[:], in_=seq[:])

        ones1_f = consts.tile([1, B], F32)
        nc.vector.memset(ones1_f[:], 1.0)
        ones1 = consts.tile([1, B], BF16)
        nc.vector.tensor_copy(ones1[:], ones1_f[:])
        bo_b = consts.tile([1, EK], BF16)
        nc.vector.tensor_copy(bo_b[:], bo_sb[:])

        pp = psum_p.tile([B, EK], F32, tag="pp")
        wo3 = wot_sb[:].rearrange("p (t e) -> p t e", e=EK)
        for t in range(NT):
            nc.tensor.matmul(
                pp[:], vta3[:, t, :], wo3[:, t, :], start=(t == 0), stop=False
            )
        nc.tensor.matmul(pp[:], ones1[:], bo_b[:], start=False, stop=True)

        p_re = sbuf.tile([P, P], F32)
        nc.vector.tensor_copy(p_re[0:B, :], pp[:, 0:P])
        nc.vector.tensor_copy(p_re[B : 2 * B, :], pp[:, P : 2 * P])

        out_sb = sbuf.tile([P, S * 128], F32)
        o3 = out_sb[:].rearrange("p (s e) -> p s e", e=P)
        q3 = seq_sb[:].rearrange("p (s e) -> p s e", e=P)
        chunks = [  # (engine, s0, s1, queue)
            (nc.gpsimd, 24, 32, nc.scalar),
            (nc.vector, 0, 12, nc.sync),
            (nc.vector, 12, 24, nc.sync),
        ]
        for eng, s0, s1, q in chunks:
            eng.tensor_add(
                o3[:, s0:s1, :], q3[:, s0:s1, :],
                p_re[:, None, :].to_broadcast((P, s1 - s0, P)),
            )
            q.dma_start(
                out=out[:, s0 * 128 : s1 * 128], in_=out_sb[:, s0 * 128 : s1 * 128]
            )
        ctx.close()
    nc.compile()
    return nc


def _get_ncs():
    if "nc1" not in _CACHE:
        _CACHE["nc1"] = _build_nc1()
        _CACHE["nc2"] = _build_nc2()
    return _CACHE["nc1"], _CACHE["nc2"]


def _run(inputs, trace=False):
    nc1, nc2 = _get_ncs()
    bf = ml_dtypes.bfloat16
    sequence = np.asarray(inputs["sequence"])
    controls = np.asarray(inputs["controls"])
    Wv = np.asarray(inputs["Wv"])
    bv = np.asarray(inputs["bv"])
    Wo = np.asarray(inputs["Wo"])
    bo = np.asarray(inputs["bo"])

    ctrl_t = np.ascontiguousarray(
        controls.transpose(2, 0, 1).reshape(D, C * B).astype(bf)
    )
    in1 = []
    for k in range(N_CORES):
        fk = slice(k * EK, (k + 1) * EK)
        in1.append(
            {
                "ctrl": ctrl_t,
                "wvt": np.ascontiguousarray(Wv[fk, :].T.astype(bf)),
                "bv": np.ascontiguousarray(bv[fk]),
            }
        )
    res1 = run_bass_kernel_spmd(nc1, in1, list(range(N_CORES)), trace=trace)

    vta = np.concatenate([np.asarray(res1.results[k]["vt"]) for k in range(N_CORES)])
    vta = np.ascontiguousarray(vta)  # (D, B) bf16

    in2 = []
    for k in range(N_CORES):
        ek = slice(k * EK, (k + 1) * EK)
        in2.append(
            {
                "vta": vta,
                "wot": np.ascontiguousarray(Wo[ek, :].T.astype(bf)),
                "bo": np.ascontiguousarray(bo[ek]),
                "seq": np.ascontiguousarray(
                    sequence[:, :, ek]
                    .reshape(B, S, 2, 128)
                    .transpose(2, 0, 1, 3)
                    .reshape(128, S * 128)
                    .astype(bf)
                ),
            }
        )
    res2 = run_bass_kernel_spmd(nc2, in2, list(range(N_CORES)), trace=trace)

    out = np.empty((B, S, D), dtype=np.float32)
    for k in range(N_CORES):
        out[:, :, k * EK : (k + 1) * EK] = (
            res2.results[k]["out"]
            .reshape(2, B, S, 128)
            .transpose(1, 2, 0, 3)
            .reshape(B, S, EK)
        )
    return out, (res1, res2)


def kernel(**inputs):
    out, _ = _run(inputs)
    return out


# revision 16
# speedup vs baseline: 1.3209x; 1.0323x over previous
"""Trainium2 Bass kernel for nn_CompactControlAttention.

The module's attention is degenerate: softmax over a size-1 axis is exactly
1.0, so queries/keys (Wq, bq, Wk, bk) never affect the output:

    out[b, s, :] = sequence[b, s, :] + p[b, :]
    p = (sum_c controls[c]) @ Wv.T @ Wo.T + C * (bv @ Wo.T + bo)

Sharding: tensor-parallel over the hidden feature dim f of v = cs @ Wv.T
and over the output feature dim e of p. Cross-core exchange of the tiny
v.T (256KB) happens between two NEFF launches via host gather -- on-chip
collectives cost ~75us of fixed setup per execute on this stack, and HBM
is only pair-shared, so a host hop is the cheapest 8-way exchange.

NEFF-1 (per core k, ~3MB DMA):
  cs_t = sum_c controls_t[c]      (controls shipped pre-transposed, bf16)
  v_k  = cs @ Wv.T[:, fk] + C*bv  (16 bf16 matmuls, 256-wide, PSUM accum)
  vt_k = v_k.T                    (2 PE transposes) -> out [256, 64] bf16

NEFF-2 (per core k, ~4.3MB DMA; host feeds the gathered full v.T):
  p_k  = v @ Wo.T[:, ek] + bo     (16 bf16 matmuls)
  out  = seq_k + broadcast_s(p_k) (chunked DVE/GpSimd adds, piped DMA)
"""

import numpy as np
import ml_dtypes

import concourse.bass as bass
import concourse.mybir as mybir
import concourse.tile as tile
from concourse import bacc
from concourse.bass_utils import run_bass_kernel_spmd
from concourse.masks import make_identity

N_CORES = 8
D = 2048
B = 64
S = 32
C = 8
EK = D // N_CORES  # 256
NT = D // 128  # 16
F32 = mybir.dt.float32
BF16 = mybir.dt.bfloat16

_CACHE = {}


# --------------------------- NEFF-1: v.T slice ---------------------------


def _build_nc1():
    nc = bacc.Bacc("TRN2", target_bir_lowering=False, debug=False, num_devices=N_CORES)
    ctrl = nc.dram_tensor("ctrl", [D, C * B], BF16, kind="ExternalInput")
    wvt = nc.dram_tensor("wvt", [D, EK], BF16, kind="ExternalInput")  # Wv.T[:, fk]
    bv = nc.dram_tensor("bv", [EK], F32, kind="ExternalInput")
    vt_out = nc.dram_tensor("vt", [EK, B], BF16, kind="ExternalOutput")

    with tile.TileContext(nc) as tc:
        from contextlib import ExitStack

        ctx = ExitStack()
        P = 128
        consts = ctx.enter_context(tc.tile_pool(name="consts", bufs=1))
        sbuf = ctx.enter_context(tc.tile_pool(name="sbuf", bufs=1))
        psum_v = ctx.enter_context(tc.tile_pool(name="psum_v", bufs=1, space="PSUM"))
        psum_t = ctx.enter_context(tc.tile_pool(name="psum_t", bufs=1, space="PSUM"))

        # ctrl in quarters (2 per HWDGE queue) so cs adds pipeline with DMA
        ctrl_sb = sbuf.tile([P, NT * C * B], BF16)
        c3 = ctrl_sb[:].rearrange("p (t cb) -> p t cb", cb=C * B)
        Q = NT // 4  # 4 t-tiles per quarter
        for qi in range(4):
            q = nc.sync if qi % 2 == 0 else nc.scalar
            q.dma_start(
                out=c3[:, qi * Q : (qi + 1) * Q, :],
                in_=ctrl[qi * 512 : (qi + 1) * 512, :].rearrange(
                    "(t p) cb -> p t cb", p=P
                ),
            )
        wv_sb = sbuf.tile([P, NT * EK], BF16)
        wv4 = wv_sb[:].rearrange("p (q t f) -> p q (t f)", q=4, f=EK)
        for qi in range(4):
            q = nc.sync if qi % 2 == 0 else nc.scalar
            q.dma_start(
                out=wv4[:, qi, :].rearrange("p (t f) -> p t f", f=EK),
                in_=wvt[qi * 512 : (qi + 1) * 512, :].rearrange(
                    "(t p) f -> p t f", p=P
                ),
            )
        bv_sb = consts.tile([1, EK], F32)
        nc.gpsimd.dma_start(out=bv_sb[:], in_=bv[None, :])

        ident = consts.tile([P, P], F32)
        make_identity(nc, ident[:])
        ident_b = consts.tile([P, P], BF16)
        nc.vector.tensor_copy(ident_b[:], ident[:])
        ones8_f = consts.tile([1, B], F32)
        nc.vector.memset(ones8_f[:], float(C))
        ones8 = consts.tile([1, B], BF16)
        nc.vector.tensor_copy(ones8[:], ones8_f[:])
        bv_b = consts.tile([1, EK], BF16)
        nc.vector.tensor_copy(bv_b[:], bv_sb[:])

        # cs tree sum, per ctrl quarter as it lands (DVE + GpSimd split)
        c4 = ctrl_sb[:].rearrange("p (t c b) -> p t c b", c=C, b=B)
        s1 = sbuf.tile([P, NT * 4 * B], BF16)
        s1v = s1[:].rearrange("p (t c b) -> p t c b", c=4, b=B)
        s2 = sbuf.tile([P, NT * 2 * B], BF16)
        s2v = s2[:].rearrange("p (t c b) -> p t c b", c=2, b=B)
        cs = sbuf.tile([P, NT * B], BF16)
        csv = cs[:].rearrange("p (t b) -> p t b", b=B)
        for qi in range(4):
            ts = slice(qi * Q, (qi + 1) * Q)
            eng = nc.vector if qi % 2 == 0 else nc.gpsimd
            eng.tensor_add(s1v[:, ts], c4[:, ts, 0:4, :], c4[:, ts, 4:8, :])
            eng.tensor_add(s2v[:, ts], s1v[:, ts, 0:2, :], s1v[:, ts, 2:4, :])
            eng.tensor_add(csv[:, ts], s2v[:, ts, 0, :], s2v[:, ts, 1, :])

        # MM1 + bias
        pv = psum_v.tile([B, EK], F32, tag="pv")
        wv3 = wv_sb[:].rearrange("p (t f) -> p t f", f=EK)
        for t in range(NT):
            nc.tensor.matmul(
                pv[:], csv[:, t, :], wv3[:, t, :], start=(t == 0), stop=False
            )
        nc.tensor.matmul(pv[:], ones8[:], bv_b[:], start=False, stop=True)
        v = sbuf.tile([B, EK], BF16)
        nc.vector.tensor_copy(v[:], pv[:])

        # vt = v.T
        pt = psum_t.tile([P, 2 * B], BF16, tag="pt")
        for g in range(2):
            nc.tensor.transpose(
                pt[:, g * B : (g + 1) * B], v[:, g * 128 : (g + 1) * 128],
                ident_b[0:B, 0:B],
            )
        vt = sbuf.tile([P, 2 * B], BF16)
        nc.vector.tensor_copy(vt[:], pt[:])
        nc.sync.dma_start(
            out=vt_out[:].rearrange("(g p) b -> p g b", p=P),
            in_=vt[:].rearrange("p (g b) -> p g b", b=B),
        )
        ctx.close()
    nc.compile()
    return nc


# ------------------------ NEFF-2: MM2 + residual -------------------------


def _build_nc2():
    nc = bacc.Bacc("TRN2", target_bir_lowering=False, debug=False, num_devices=N_CORES)
    vta = nc.dram_tensor("vta", [D, B], BF16, kind="ExternalInput")  # full v.T
    wot = nc.dram_tensor("wot", [D, EK], BF16, kind="ExternalInput")  # Wo.T[:, ek]
    bo = nc.dram_tensor("bo", [EK], F32, kind="ExternalInput")
    seq = nc.dram_tensor("seq", [128, S * 128], BF16, kind="ExternalInput")
    out = nc.dram_tensor("out", [128, S * 128], F32, kind="ExternalOutput")

    with tile.TileContext(nc) as tc:
        from contextlib import ExitStack

        ctx = ExitStack()
        P = 128
        consts = ctx.enter_context(tc.tile_pool(name="consts", bufs=1))
        sbuf = ctx.enter_context(tc.tile_pool(name="sbuf", bufs=1))
        psum_p = ctx.enter_context(tc.tile_pool(name="psum_p", bufs=1, space="PSUM"))

        vta_sb = sbuf.tile([P, NT * B], BF16)
        vta3 = vta_sb[:].rearrange("p (t b) -> p t b", b=B)
        nc.sync.dma_start(out=vta3, in_=vta.rearrange("(t p) b -> p t b", p=P))
        # wot first on BOTH queues (halves) -- MM2's critical input
        wot_sb = sbuf.tile([P, NT * EK], BF16)
        wo_h = wot_sb[:].rearrange("p (h t e) -> p h (t e)", h=2, e=EK)
        for hi in range(2):
            q = nc.scalar if hi == 0 else nc.sync
            q.dma_start(
                out=wo_h[:, hi, :].rearrange("p (t e) -> p t e", e=EK),
                in_=wot[hi * 1024 : (hi + 1) * 1024, :].rearrange(
                    "(t p) e -> p t e", p=P
                ),
            )
        bo_sb = consts.tile([1, EK], F32)
        nc.gpsimd.dma_start(out=bo_sb[:], in_=bo[None, :])
        seq_sb = sbuf.tile([P, S * 128], BF16)
        nc.sync.dma_start(out=seq_sb[:, 0 : S * 64], in_=seq[:, 0 : S * 64])
        nc.scalar.dma_start(out=seq_sb[:, S * 64 :], in_=seq[:, S * 64 :])

        ones1_f = consts.tile([1, B], F32)
        nc.vector.memset(ones1_f[:], 1.0)
        ones1 = consts.tile([1, B], BF16)
        nc.vector.tensor_copy(ones1[:], ones1_f[:])
        bo_b = consts.tile([1, EK], BF16)
        nc.vector.tensor_copy(bo_b[:], bo_sb[:])

        pp = psum_p.tile([B, EK], F32, tag="pp")
        wo3 = wot_sb[:].rearrange("p (t e) -> p t e", e=EK)
        for t in range(NT):
            nc.tensor.matmul(
                pp[:], vta3[:, t, :], wo3[:, t, :], start=(t == 0), stop=False
            )
        nc.tensor.matmul(pp[:], ones1[:], bo_b[:], start=False, stop=True)

        p_re = sbuf.tile([P, P], F32)
        nc.vector.tensor_copy(p_re[0:B, :], pp[:, 0:P])
        nc.vector.tensor_copy(p_re[B : 2 * B, :], pp[:, P : 2 * P])

        out_sb = sbuf.tile([P, S * 128], F32)
        o3 = out_sb[:].rearrange("p (s e) -> p s e", e=P)
        q3 = seq_sb[:].rearrange("p (s e) -> p s e", e=P)
        chunks = [  # (engine, s0, s1, queue)
            (nc.gpsimd, 28, 32, nc.scalar),
            (nc.vector, 0, 10, nc.sync),
            (nc.vector, 10, 20, nc.sync),
            (nc.vector, 20, 28, nc.scalar),
        ]
        for eng, s0, s1, q in chunks:
            eng.tensor_add(
                o3[:, s0:s1, :], q3[:, s0:s1, :],
                p_re[:, None, :].to_broadcast((P, s1 - s0, P)),
            )
            q.dma_start(
                out=out[:, s0 * 128 : s1 * 128], in_=out_sb[:, s0 * 128 : s1 * 128]
            )
        ctx.close()
    nc.compile()
    return nc


def _get_ncs():
    if "nc1" not in _CACHE:
        _CACHE["nc1"] = _build_nc1()
        _CACHE["nc2"] = _build_nc2()
    return _CACHE["nc1"], _CACHE["nc2"]


def _run(inputs, trace=False):
    nc1, nc2 = _get_ncs()
    bf = ml_dtypes.bfloat16
    sequence = np.asarray(inputs["sequence"])
    controls = np.asarray(inputs["controls"])
    Wv = np.asarray(inputs["Wv"])
    bv = np.asarray(inputs["bv"])
    Wo = np.asarray(inputs["Wo"])
    bo = np.asarray(inputs["bo"])

    ctrl_t = np.ascontiguousarray(
        controls.transpose(2, 0, 1).reshape(D, C * B).astype(bf)
    )
    in1 = []
    for k in range(N_CORES):
        fk = slice(k * EK, (k + 1) * EK)
        in1.append(
            {
                "ctrl": ctrl_t,
                "wvt": np.ascontiguousarray(Wv[fk, :].T.astype(bf)),
                "bv": np.ascontiguousarray(bv[fk]),
            }
        )
    res1 = run_bass_kernel_spmd(nc1, in1, list(range(N_CORES)), trace=trace)

    vta = np.concatenate([np.asarray(res1.results[k]["vt"]) for k in range(N_CORES)])
    vta = np.ascontiguousarray(vta)  # (D, B) bf16

    in2 = []
    for k in range(N_CORES):
        ek = slice(k * EK, (k + 1) * EK)
        in2.append(
            {
                "vta": vta,
                "wot": np.ascontiguousarray(Wo[ek, :].T.astype(bf)),
                "bo": np.ascontiguousarray(bo[ek]),
                "seq": np.ascontiguousarray(
                    sequence[:, :, ek]
                    .reshape(B, S, 2, 128)
                    .transpose(2, 0, 1, 3)
                    .reshape(128, S * 128)
                    .astype(bf)
                ),
            }
        )
    res2 = run_bass_kernel_spmd(nc2, in2, list(range(N_CORES)), trace=trace)

    out = np.empty((B, S, D), dtype=np.float32)
    for k in range(N_CORES):
        out[:, :, k * EK : (k + 1) * EK] = (
            res2.results[k]["out"]
            .reshape(2, B, S, 128)
            .transpose(1, 2, 0, 3)
            .reshape(B, S, EK)
        )
    return out, (res1, res2)


def kernel(**inputs):
    out, _ = _run(inputs)
    return out
